# revision 8
# baseline (speedup 1.0000x reference)
"""Trainium2 Bass kernel for nn_CrossAttention (masked dual-softmax cross attention).

Per-batch math (reference):
    Ma = A @ Wa + ba; Mb = B @ Wb + bb         (ba = bb = 0 in this problem)
    S  = (Ma @ Mb^T) / sqrt(D), masked to -1e9 where mask_a[i]*mask_b[j] == 0
    att_a  = softmax(S, axis=-1); att_bT = softmax(S, axis=1)
    out_a = att_bT @ B + A;  out_b = att_a^T @ A + B

Sharding: data-parallel over batch B=8 across the 8 NeuronCores (one batch
element per core, weights replicated, no collectives).

Per-core algorithm. All large GEMMs run in fp8e4m3 with DoubleRow perf mode
(two K-rows per PE cell, K=256 per matmul) accumulating in fp32 PSUM:
    G   = KG * scale * Wb @ Wa^T   (e x d, fp8, prescaled so fp8 doesn't
                                    denormal-flush; scale = 1/sqrt(D))
    HT  = (KH/KG) * G^T @ B^T      (d x j, fp8, = KH * scale * true H^T)
    S   = AT^T @ HT  (PSUM = KH * S_true); E = exp(S_psum / KH) via ACT,
          then one fused DVE op: E *= mb_row (zero masked cols, fp8 out)
          with accum_out giving the row sums Za in the same pass.
    out_b = E^T @ (A * KS*ma/Za) / KS + cA + B   (cA = uniform correction for
          fully-masked rows, folded into the PSUM group as a K=1 rank-1
          matmul; the +B residual and 1/KS fold into one fused DVE op)
    ... and symmetrically for out_a from E2 = exp(S^T) * ma_row.
KS=512 rescales A*ma/Za (~6e-4) into fp8e4m3's representable range.
"""

import math

import numpy as np

import concourse.bass as bass
import concourse.mybir as mybir
import concourse.tile as tile
from concourse.masks import make_identity

F32 = mybir.dt.float32
BF16 = mybir.dt.bfloat16
F8 = mybir.dt.float8e4
I32 = mybir.dt.int32
P = 128
SC = 512            # matmul free-dim chunk (one PSUM bank of fp32)

AX = mybir.AxisListType
OP = mybir.AluOpType
AF = mybir.ActivationFunctionType
DRMODE = mybir.MatmulPerfMode.DoubleRow

KG = 64.0           # G prescale (keeps fp8 G out of denormal range)
KH = 8.0            # HT prescale; ACT exp applies 1/KH for free
KS = 64.0           # At_s/Bt_s prescale (A*ma/Za would flush in fp8)
CEXP = 4.0          # E = exp(S - CEXP): keeps exp below fp8e4m3's 240 max
                    # (softmax is shift-invariant; Za shrinks by e^-CEXP)


def build_nc(La=2048, Lb=2048, D=512, split_waits=True):
    H = D
    NTI, NTJ, DT = La // P, Lb // P, D // P
    NPI, NPJ, DP = NTI // 2, NTJ // 2, DT // 2
    SBW = min(1024, Lb)          # S-psum tile width (2 banks)
    scale = 1.0 / math.sqrt(D)

    nc = bass.Bass()
    A_d = nc.declare_dram_parameter("input_a", [La, D], F32, isOutput=False)
    B_d = nc.declare_dram_parameter("input_b", [Lb, D], F32, isOutput=False)
    ma_d = nc.declare_dram_parameter("mask_a", [La], I32, isOutput=False)
    mb_d = nc.declare_dram_parameter("mask_b", [Lb], I32, isOutput=False)
    Wa_d = nc.declare_dram_parameter("Wa", [D, H], F32, isOutput=False)
    Wb_d = nc.declare_dram_parameter("Wb", [D, H], F32, isOutput=False)
    oa_d = nc.declare_dram_parameter("out_a", [La, D], F32, isOutput=True)
    ob_d = nc.declare_dram_parameter("out_b", [Lb, D], F32, isOutput=True)

    A3 = A_d.rearrange("(t p) d -> p t d", p=P)
    B3 = B_d.rearrange("(t p) d -> p t d", p=P)
    Wa3 = Wa_d.rearrange("(t p) h -> p t h", p=P)
    Wb3 = Wb_d.rearrange("(t p) h -> p t h", p=P)
    oa3 = oa_d.rearrange("(t p) d -> p t d", p=P)
    ob3 = ob_d.rearrange("(t p) d -> p t d", p=P)

    with tile.TileContext(nc) as tc:
        with (
            tc.tile_pool(name="const", bufs=1) as constp,
            tc.tile_pool(name="big", bufs=1) as bigp,
            tc.tile_pool(name="tmp", bufs=4) as tmpp,
            tc.tile_pool(name="tmp1", bufs=1) as tmp1p,
            tc.tile_pool(name="eraw", bufs=2) as erawp,
            tc.tile_pool(name="io", bufs=3) as iop,
            tc.tile_pool(name="oio", bufs=3) as oiop,
            tc.tile_pool(name="ps_s", bufs=2, space="PSUM") as ps_s,
            tc.tile_pool(name="ps_t", bufs=2, space="PSUM") as ps_t,
            tc.tile_pool(name="ps_o", bufs=2, space="PSUM") as ps_o,
        ):
            # ---------------- constants ----------------
            identb = constp.tile([P, P], BF16, tag="identb")
            make_identity(nc, identb)
            ident8 = constp.tile([P, P], F8, tag="ident8")
            make_identity(nc, ident8)
            ones1 = constp.tile([1, P], BF16, tag="ones1")
            nc.vector.memset(ones1, 1.0)
            onespp = constp.tile([P, P], BF16, tag="onespp")
            nc.vector.memset(onespp, 1.0)
            negc = constp.tile([P, 1], F32, tag="negc")
            nc.vector.memset(negc, -CEXP)

            # ---------------- masks ----------------
            LM = max(La, Lb)
            # {0,1} mask rows broadcast across all 128 partitions (bf16);
            # multiplied into E by the fused mask+rowsum DVE op.
            Mbb = constp.tile([P, Lb], BF16, tag="Mbb")
            Mab = constp.tile([P, La], BF16, tag="Mab")
            for m_d, L, bc in ((mb_d, Lb, Mbb), (ma_d, La, Mab)):
                mri = tmp1p.tile([1, LM], I32, tag="mrow_i")
                nc.sync.dma_start(mri[:, :L], m_d.rearrange("(a j) -> a j", a=1))
                mrf = tmp1p.tile([1, LM], BF16, tag="mrow_f")
                nc.vector.tensor_copy(mrf, mri)
                for c in range(L // SC):
                    pm = ps_t.tile([P, SC], F32, tag="ps_t")
                    nc.tensor.matmul(pm, ones1, mrf[:, c * SC:(c + 1) * SC],
                                     start=True, stop=True)
                    nc.vector.tensor_copy(bc[:, c * SC:(c + 1) * SC], pm)

            mcol_i = tmp1p.tile([P, NTI + NTJ], I32, tag="mcol_i")
            nc.sync.dma_start(mcol_i[:, :NTI], ma_d.rearrange("(t p) -> p t", p=P))
            nc.sync.dma_start(mcol_i[:, NTI:], mb_d.rearrange("(t p) -> p t", p=P))
            mcol_f = constp.tile([P, NTI + NTJ], F32, tag="mcol_f")
            nc.vector.tensor_copy(mcol_f, mcol_i)
            macol = mcol_f[:, :NTI]
            mbcol = mcol_f[:, NTI:]
            # ucol = KS*(1-m)/L  (uniform-softmax weight, KS-prescaled)
            ucol = constp.tile([P, NTI + NTJ], F32, tag="ucol")
            nc.vector.tensor_scalar(
                ucol[:, :NTI], macol, 1.0, -KS / Lb, OP.subtract, OP.mult)
            nc.vector.tensor_scalar(
                ucol[:, NTI:], mbcol, 1.0, -KS / La, OP.subtract, OP.mult)

            # ---------------- W: load, cast, transpose ----------------
            WaT = tmpp.tile([P, DT, D], BF16, tag="sc4k")
            WbT = tmpp.tile([P, DT, D], BF16, tag="sc4k")
            for W3, WT in ((Wa3, WaT), (Wb3, WbT)):
                wbf = tmpp.tile([P, DT, H], BF16, tag="sc4k")
                for dt in range(DT):
                    s = iop.tile([P, H], F32, tag="io_in")
                    nc.sync.dma_start(s, W3[:, dt, :])
                    nc.vector.tensor_copy(wbf[:, dt, :], s)
                for ht in range(DT):
                    pst = ps_t.tile([P, SC], BF16, tag="ps_t")
                    for dt in range(DT):
                        nc.tensor.transpose(
                            pst[:, dt * P:(dt + 1) * P],
                            wbf[:, dt, ht * P:(ht + 1) * P], identb)
                    nc.scalar.copy(WT[:, ht, :], pst[:, :DT * P])

            # -------- G = KG * scale * Wb @ Wa^T  (e x d, fp8) --------
            G = constp.tile([P, DT, D], F8, tag="G")
            for et in range(DT):
                pg = ps_t.tile([P, SC], F32, tag="ps_t")
                for ht in range(DT):
                    nc.tensor.matmul(
                        pg[:, :D], WbT[:, ht, et * P:(et + 1) * P], WaT[:, ht, :],
                        start=(ht == 0), stop=(ht == DT - 1))
                nc.scalar.mul(G[:, et, :], pg[:, :D], KG * scale)

            # ------- B: load fp32 (kept for residual), cast fp8, transpose ----
            B_bf = bigp.tile([P, NTJ, D], BF16, tag="B_bf")
            B_f8 = bigp.tile([P, NTJ, D], F8, tag="B_f8")
            BT = bigp.tile([P, DT, Lb], F8, tag="BT")
            for g in range(Lb // SC):
                for k in range(SC // P):
                    t = g * (SC // P) + k
                    s = iop.tile([P, D], F32, tag="io_in")
                    nc.sync.dma_start(s, B3[:, t, :])
                    nc.vector.tensor_copy(B_bf[:, t, :], s)
                    nc.vector.tensor_copy(B_f8[:, t, :], s)
                for dt in range(DT):
                    pst = ps_t.tile([P, SC, 2], F8, tag="ps_t")
                    for k in range(SC // P):
                        jt = g * (SC // P) + k
                        nc.tensor.transpose(
                            pst[:, k * P:(k + 1) * P, 0],
                            B_f8[:, jt, dt * P:(dt + 1) * P], ident8)
                    nc.scalar.copy(BT[:, dt, g * SC:(g + 1) * SC], pst[:, :, 0])

            # ---------------- HT = (KH/KG) * G^T @ BT  (d x j, fp8) ----------
            HT = bigp.tile([P, DT, Lb], F8, tag="HT")
            for dt in range(DT):
                for jc in range(Lb // SC):
                    ph = ps_t.tile([P, SC], F32, tag="ps_t")
                    for q in range(DP):
                        nc.tensor.matmul(
                            ph, G[:, 2 * q:2 * q + 2, dt * P:(dt + 1) * P],
                            BT[:, 2 * q:2 * q + 2, jc * SC:(jc + 1) * SC],
                            start=(q == 0), stop=(q == DP - 1),
                            perf_mode=DRMODE)
                    nc.scalar.mul(HT[:, dt, jc * SC:(jc + 1) * SC], ph, KH / KG)

            # ------- A: load fp32, cast fp8, transpose to AT ----
            A_f32 = bigp.tile([P, NTI, D], F32, tag="A_f32")
            A_f8 = bigp.tile([P, NTI, D], F8, tag="A_f8")
            AT = bigp.tile([P, DT, La], F8, tag="AT")
            for g in range(La // SC):
                for k in range(SC // P):
                    t = g * (SC // P) + k
                    nc.sync.dma_start(A_f32[:, t, :], A3[:, t, :])
                    nc.vector.tensor_copy(A_f8[:, t, :], A_f32[:, t, :])
                for dt in range(DT):
                    pst = ps_t.tile([P, SC, 2], F8, tag="ps_t")
                    for k in range(SC // P):
                        it = g * (SC // P) + k
                        nc.tensor.transpose(
                            pst[:, k * P:(k + 1) * P, 0],
                            A_f8[:, it, dt * P:(dt + 1) * P], ident8)
                    nc.scalar.copy(AT[:, dt, g * SC:(g + 1) * SC], pst[:, :, 0])

            # column-replicated KS*(1-m)/L tiles, lhsT for the cA/cB GEMMs
            uac = tmpp.tile([P, NTI, P], F8, tag="sc4k8")
            ubc = tmpp.tile([P, NTJ, P], F8, tag="sc4k8")
            for t in range(NTI):
                nc.vector.tensor_scalar_mul(uac[:, t, :], onespp, ucol[:, t:t + 1])
            for t in range(NTJ):
                nc.vector.tensor_scalar_mul(
                    ubc[:, t, :], onespp, ucol[:, NTI + t:NTI + t + 1])

            # ------------- cA / cB rank-1 corrections (KS-prescaled) ---------
            cA = constp.tile([P, D], BF16, tag="cA")
            pc = ps_o.tile([P, D], F32, tag="ps_o")
            for pt in range(NPI):
                nc.tensor.matmul(pc, uac[:, 2 * pt:2 * pt + 2, :],
                                 A_f8[:, 2 * pt:2 * pt + 2, :],
                                 start=(pt == 0), stop=(pt == NPI - 1),
                                 perf_mode=DRMODE)
            nc.vector.tensor_copy(cA, pc)
            cB = constp.tile([P, D], BF16, tag="cB")
            pc = ps_o.tile([P, D], F32, tag="ps_o")
            for pt in range(NPJ):
                nc.tensor.matmul(pc, ubc[:, 2 * pt:2 * pt + 2, :],
                                 B_f8[:, 2 * pt:2 * pt + 2, :],
                                 start=(pt == 0), stop=(pt == NPJ - 1),
                                 perf_mode=DRMODE)
            nc.vector.tensor_copy(cB, pc)

            # ======= phase 1: E = mb_j * exp(S)  (i x j), Za row sums =======
            nblk = Lb // SBW
            E = bigp.tile([P, NTI, Lb], F8, tag="E")
            Za = constp.tile([P, NTI], F32, tag="Za")
            for it in range(NTI):
                eraw = erawp.tile([P, Lb], BF16, tag="eraw")
                for blk in range(nblk):
                    ps = ps_s.tile([P, SBW], F32, tag="ps_s")
                    for c in range(SBW // SC):
                        jc = blk * (SBW // SC) + c
                        sl = slice(c * SC, (c + 1) * SC)
                        for q in range(DP):
                            nc.tensor.matmul(
                                ps[:, sl],
                                AT[:, 2 * q:2 * q + 2, it * P:(it + 1) * P],
                                HT[:, 2 * q:2 * q + 2, jc * SC:(jc + 1) * SC],
                                start=(q == 0), stop=(q == DP - 1),
                                perf_mode=DRMODE)
                    nc.scalar.activation(
                        eraw[:, blk * SBW:(blk + 1) * SBW], ps, AF.Exp,
                        scale=1.0 / KH, bias=negc)
                # fused: E = eraw * mb (fp8), Za = rowsum(E) — one DVE pass
                nc.vector.scalar_tensor_tensor(
                    E[:, it, :], eraw, 1.0, Mbb, OP.bypass, OP.mult,
                    accum_out=Za[:, it:it + 1])

            # ---- out_b = E^T @ (A * KS*ma/Za) / KS + cA + B ----
            qa = constp.tile([P, NTI], F32, tag="qa")
            nc.vector.reciprocal(qa, Za)
            nc.vector.scalar_tensor_tensor(qa, qa, KS, macol, OP.mult, OP.mult)
            At_s = bigp.tile([P, NTI, D], F8, tag="ATS")
            for t in range(NTI):
                nc.vector.tensor_scalar_mul(At_s[:, t, :], A_f32[:, t, :],
                                            qa[:, t:t + 1])
            for jt in range(NTJ):
                po = ps_o.tile([P, D], F32, tag="ps_o")
                nc.tensor.matmul(po, ones1, cA[0:1, :], start=True, stop=False)
                for pt in range(NPI):
                    nc.tensor.matmul(po, E[:, 2 * pt:2 * pt + 2,
                                          jt * P:(jt + 1) * P],
                                     At_s[:, 2 * pt:2 * pt + 2, :],
                                     start=False, stop=(pt == NPI - 1),
                                     perf_mode=DRMODE)
                ot = oiop.tile([P, D], F32, tag="io_out")
                nc.vector.scalar_tensor_tensor(
                    ot, po, 1.0 / KS, B_bf[:, jt, :], OP.mult, OP.add)
                nc.sync.dma_start(ob3[:, jt, :], ot)

            # ======= phase 2: E2 = ma_i * exp(S^T)  (j x i), Zb row sums ====
            nblk2 = La // SBW
            E2 = bigp.tile([P, NTJ, La], F8, tag="E")
            Zb = constp.tile([P, NTJ], F32, tag="Zb")
            for jt in range(NTJ):
                eraw = erawp.tile([P, La], BF16, tag="eraw")
                for blk in range(nblk2):
                    ps = ps_s.tile([P, SBW], F32, tag="ps_s")
                    for c in range(SBW // SC):
                        ic = blk * (SBW // SC) + c
                        sl = slice(c * SC, (c + 1) * SC)
                        for q in range(DP):
                            nc.tensor.matmul(
                                ps[:, sl],
                                HT[:, 2 * q:2 * q + 2, jt * P:(jt + 1) * P],
                                AT[:, 2 * q:2 * q + 2, ic * SC:(ic + 1) * SC],
                                start=(q == 0), stop=(q == DP - 1),
                                perf_mode=DRMODE)
                    nc.scalar.activation(
                        eraw[:, blk * SBW:(blk + 1) * SBW], ps, AF.Exp,
                        scale=1.0 / KH, bias=negc)
                nc.vector.scalar_tensor_tensor(
                    E2[:, jt, :], eraw, 1.0, Mab, OP.bypass, OP.mult,
                    accum_out=Zb[:, jt:jt + 1])

            # ---- out_a = E2^T @ (B * KS*mb/Zb) / KS + cB + A ----
            rb = constp.tile([P, NTJ], F32, tag="rb")
            nc.vector.reciprocal(rb, Zb)
            nc.vector.scalar_tensor_tensor(rb, rb, KS, mbcol, OP.mult, OP.mult)
            Bt_s = bigp.tile([P, NTJ, D], F8, tag="ATS")
            for t in range(NTJ):
                nc.vector.tensor_scalar_mul(Bt_s[:, t, :], B_bf[:, t, :],
                                            rb[:, t:t + 1])
            for it in range(NTI):
                po = ps_o.tile([P, D], F32, tag="ps_o")
                nc.tensor.matmul(po, ones1, cB[0:1, :], start=True, stop=False)
                for pt in range(NPJ):
                    nc.tensor.matmul(po, E2[:, 2 * pt:2 * pt + 2,
                                           it * P:(it + 1) * P],
                                     Bt_s[:, 2 * pt:2 * pt + 2, :],
                                     start=False, stop=(pt == NPJ - 1),
                                     perf_mode=DRMODE)
                ot = oiop.tile([P, D], F32, tag="io_out")
                nc.vector.scalar_tensor_tensor(
                    ot, po, 1.0 / KS, A_f32[:, it, :], OP.mult, OP.add)
                nc.sync.dma_start(oa3[:, it, :], ot)

    if split_waits:
        _split_multi_waits(nc)
    return nc


def _split_multi_waits(nc):
    """This toolchain's walrus encodes at most ONE sync wait per engine
    instruction ("Too many sync wait commands"). Hoist all but one wait of
    each offending instruction onto injected same-engine NoOps immediately
    before it: sequential waits on one engine are AND semantics."""
    nop_id = 0
    for bb in nc.main_func.blocks:
        il = bb.instructions
        idx = 0
        while idx < len(il):
            ins = il[idx]
            si = ins.sync_info
            if si is not None and si.on_wait and len(si.on_wait) > 1:
                waits = list(si.on_wait)
                ins.sync_info = mybir.SyncInfo(
                    on_wait=[waits[-1]], on_update=list(si.on_update or []))
                for w in waits[:-1]:
                    nop = mybir.InstNoOp(
                        name=f"I-waitnop-{nop_id}", ins=[], outs=[],
                        engine=ins.engine,
                        sync_info=mybir.SyncInfo(on_wait=[w], on_update=[]))
                    nop_id += 1
                    il.insert(idx, nop)
                    idx += 1
            idx += 1


_NC_CACHE = {}


def _get_nc(La=2048, Lb=2048, D=512):
    key = (La, Lb, D)
    if key not in _NC_CACHE:
        _NC_CACHE[key] = build_nc(La, Lb, D)
    return _NC_CACHE[key]


def _shard(inputs):
    Bn = inputs["input_a"].shape[0]
    names = ("input_a", "input_b", "mask_a", "mask_b")
    in_maps = []
    for b in range(Bn):
        m = {n: np.ascontiguousarray(inputs[n][b]) for n in names}
        m["Wa"] = np.ascontiguousarray(inputs["Wa"])
        m["Wb"] = np.ascontiguousarray(inputs["Wb"])
        in_maps.append(m)
    return in_maps


def kernel(**inputs):
    from concourse.bass_utils import run_bass_kernel_spmd

    inputs = {k: np.asarray(v) for k, v in inputs.items()}
    # the kernel folds the (identically-zero) biases away
    assert not inputs["ba"].any() and not inputs["bb"].any()
    Bn, La, D = inputs["input_a"].shape
    Lb = inputs["input_b"].shape[1]
    nc = _get_nc(La, Lb, D)
    in_maps = _shard(inputs)
    res = run_bass_kernel_spmd(nc, in_maps, core_ids=list(range(Bn))).results
    out_a = np.stack([res[b]["out_a"] for b in range(Bn)])
    out_b = np.stack([res[b]["out_b"] for b in range(Bn)])
    return out_a, out_b


# revision 12
# speedup vs baseline: 1.0216x; 1.0216x over previous
"""Trainium2 Bass kernel for nn_CrossAttention (masked dual-softmax cross attention).

Per-batch math (reference):
    Ma = A @ Wa + ba; Mb = B @ Wb + bb         (ba = bb = 0 in this problem)
    S  = (Ma @ Mb^T) / sqrt(D), masked to -1e9 where mask_a[i]*mask_b[j] == 0
    att_a  = softmax(S, axis=-1); att_bT = softmax(S, axis=1)
    out_a = att_bT @ B + A;  out_b = att_a^T @ A + B

Sharding: data-parallel over batch B=8 across the 8 NeuronCores (one batch
element per core, weights replicated, no collectives).

Per-core algorithm. All large GEMMs run in fp8e4m3 with DoubleRow perf mode
(two K-rows per PE cell, K=256 per matmul) accumulating in fp32 PSUM:
    G   = KG * scale * Wb @ Wa^T   (e x d, fp8, prescaled so fp8 doesn't
                                    denormal-flush; scale = 1/sqrt(D))
    HT  = (KH/KG) * G^T @ B^T      (d x j, fp8, = KH * scale * true H^T)
    S   = AT^T @ HT  (PSUM = KH * S_true); E = exp(S_psum / KH) via ACT,
          then one fused DVE op: E *= mb_row (zero masked cols, fp8 out)
          with accum_out giving the row sums Za in the same pass.
    out_b = E^T @ (A * KS*ma/Za) / KS + cA + B   (cA = uniform correction for
          fully-masked rows, folded into the PSUM group as a K=1 rank-1
          matmul; the +B residual and 1/KS fold into one fused DVE op)
    ... and symmetrically for out_a from E2 = exp(S^T) * ma_row.
KS=512 rescales A*ma/Za (~6e-4) into fp8e4m3's representable range.
"""

import math

import numpy as np

import concourse.bass as bass
import concourse.mybir as mybir
import concourse.tile as tile
from concourse.masks import make_identity

F32 = mybir.dt.float32
BF16 = mybir.dt.bfloat16
F8 = mybir.dt.float8e4
I32 = mybir.dt.int32
P = 128
SC = 512            # matmul free-dim chunk (one PSUM bank of fp32)

AX = mybir.AxisListType
OP = mybir.AluOpType
AF = mybir.ActivationFunctionType
DRMODE = mybir.MatmulPerfMode.DoubleRow

KG = 64.0           # G prescale (keeps fp8 G out of denormal range)
KH = 8.0            # HT prescale; ACT exp applies 1/KH for free
KS = 64.0           # At_s/Bt_s prescale (A*ma/Za would flush in fp8)
CEXP = 4.0          # E = exp(S - CEXP): keeps exp below fp8e4m3's 240 max
                    # (softmax is shift-invariant; Za shrinks by e^-CEXP)


def build_nc(La=2048, Lb=2048, D=512, split_waits=True):
    H = D
    NTI, NTJ, DT = La // P, Lb // P, D // P
    NPI, NPJ, DP = NTI // 2, NTJ // 2, DT // 2
    SBW = min(1024, Lb)          # S-psum tile width (2 banks)
    scale = 1.0 / math.sqrt(D)

    nc = bass.Bass()
    A_d = nc.declare_dram_parameter("input_a", [La, D], F32, isOutput=False)
    B_d = nc.declare_dram_parameter("input_b", [Lb, D], F32, isOutput=False)
    ma_d = nc.declare_dram_parameter("mask_a", [La], I32, isOutput=False)
    mb_d = nc.declare_dram_parameter("mask_b", [Lb], I32, isOutput=False)
    Wa_d = nc.declare_dram_parameter("Wa", [D, H], F32, isOutput=False)
    Wb_d = nc.declare_dram_parameter("Wb", [D, H], F32, isOutput=False)
    oa_d = nc.declare_dram_parameter("out_a", [La, D], F32, isOutput=True)
    ob_d = nc.declare_dram_parameter("out_b", [Lb, D], F32, isOutput=True)

    A3 = A_d.rearrange("(t p) d -> p t d", p=P)
    B3 = B_d.rearrange("(t p) d -> p t d", p=P)
    Wa3 = Wa_d.rearrange("(t p) h -> p t h", p=P)
    Wb3 = Wb_d.rearrange("(t p) h -> p t h", p=P)
    oa3 = oa_d.rearrange("(t p) d -> p t d", p=P)
    ob3 = ob_d.rearrange("(t p) d -> p t d", p=P)

    with tile.TileContext(nc) as tc:
        with (
            tc.tile_pool(name="const", bufs=1) as constp,
            tc.tile_pool(name="big", bufs=1) as bigp,
            tc.tile_pool(name="tmp", bufs=3) as tmpp,
            tc.tile_pool(name="ucp", bufs=2) as ucp,
            tc.tile_pool(name="tmp1", bufs=1) as tmp1p,
            tc.tile_pool(name="eraw", bufs=3) as erawp,
            tc.tile_pool(name="io", bufs=2) as iop,
            tc.tile_pool(name="oio", bufs=3) as oiop,
            tc.tile_pool(name="ps_s", bufs=2, space="PSUM") as ps_s,
            tc.tile_pool(name="ps_t", bufs=2, space="PSUM") as ps_t,
            tc.tile_pool(name="ps_o", bufs=2, space="PSUM") as ps_o,
        ):
            # ---------------- constants ----------------
            identb = constp.tile([P, P], BF16, tag="identb")
            make_identity(nc, identb)
            ident8 = constp.tile([P, P], F8, tag="ident8")
            make_identity(nc, ident8)
            ones1 = constp.tile([1, P], BF16, tag="ones1")
            nc.vector.memset(ones1, 1.0)
            onespp = constp.tile([P, P], BF16, tag="onespp")
            nc.vector.memset(onespp, 1.0)
            negc = constp.tile([P, 1], F32, tag="negc")
            nc.vector.memset(negc, -CEXP)

            # ---------------- mask DMAs (tiny; issue first) ----------------
            LM = max(La, Lb)
            mri_b = tmp1p.tile([1, LM], I32, tag="mrow_ib")
            nc.sync.dma_start(mri_b[:, :Lb], mb_d.rearrange("(a j) -> a j", a=1))
            mri_a = tmp1p.tile([1, LM], I32, tag="mrow_ia")
            nc.sync.dma_start(mri_a[:, :La], ma_d.rearrange("(a j) -> a j", a=1))
            mcol_i = tmp1p.tile([P, NTI + NTJ], I32, tag="mcol_i")
            nc.sync.dma_start(mcol_i[:, :NTI], ma_d.rearrange("(t p) -> p t", p=P))
            nc.sync.dma_start(mcol_i[:, NTI:], mb_d.rearrange("(t p) -> p t", p=P))

            # ---------------- W: load, cast, transpose ----------------
            WaT = tmpp.tile([P, DT, D], BF16, tag="sc4k")
            WbT = tmpp.tile([P, DT, D], BF16, tag="sc4k")
            for W3, WT in ((Wa3, WaT), (Wb3, WbT)):
                wbf = tmpp.tile([P, DT, H], BF16, tag="sc4k")
                for dt in range(DT):
                    s = iop.tile([P, H], F32, tag="io_w")
                    nc.sync.dma_start(s, W3[:, dt, :])
                    nc.vector.tensor_copy(wbf[:, dt, :], s)
                for ht in range(DT):
                    pst = ps_t.tile([P, SC], BF16, tag="ps_t")
                    for dt in range(DT):
                        nc.tensor.transpose(
                            pst[:, dt * P:(dt + 1) * P],
                            wbf[:, dt, ht * P:(ht + 1) * P], identb)
                    nc.scalar.copy(WT[:, ht, :], pst[:, :DT * P])

            # -------- G = KG * scale * Wb @ Wa^T  (e x d, fp8) --------
            G = constp.tile([P, DT, D], F8, tag="G")
            for et in range(DT):
                pg = ps_t.tile([P, SC], F32, tag="ps_t")
                for ht in range(DT):
                    nc.tensor.matmul(
                        pg[:, :D], WbT[:, ht, et * P:(et + 1) * P], WaT[:, ht, :],
                        start=(ht == 0), stop=(ht == DT - 1))
                nc.scalar.mul(G[:, et, :], pg[:, :D], KG * scale)

            # --- B: load, cast (bf16 + fp8), transpose; HT chunk per group ---
            # HT = (KH/KG) * G^T @ BT  (d x j, fp8); chunk jc only needs
            # B-group jc, so HT GEMMs interleave with the load pipeline.
            B_bf = bigp.tile([P, NTJ, D], BF16, tag="B_bf")
            B_f8 = bigp.tile([P, NTJ, D], F8, tag="B_f8")
            BT = bigp.tile([P, DT, Lb], F8, tag="BT")
            HT = bigp.tile([P, DT, Lb], F8, tag="HT")
            for g in range(Lb // SC):
                for k in range(SC // P):
                    t = g * (SC // P) + k
                    s = iop.tile([P, D], F32, tag="io_b")
                    nc.sync.dma_start(s, B3[:, t, :])
                    nc.vector.tensor_copy(B_bf[:, t, :], s)
                    nc.vector.tensor_copy(B_f8[:, t, :], s)
                for dt in range(DT):
                    pst = ps_t.tile([P, SC, 2], F8, tag="ps_t")
                    for k in range(SC // P):
                        jt = g * (SC // P) + k
                        nc.tensor.transpose(
                            pst[:, k * P:(k + 1) * P, 0],
                            B_f8[:, jt, dt * P:(dt + 1) * P], ident8)
                    nc.scalar.copy(BT[:, dt, g * SC:(g + 1) * SC], pst[:, :, 0])
                for dt in range(DT):
                    ph = ps_t.tile([P, SC], F32, tag="ps_t")
                    for q in range(DP):
                        nc.tensor.matmul(
                            ph, G[:, 2 * q:2 * q + 2, dt * P:(dt + 1) * P],
                            BT[:, 2 * q:2 * q + 2, g * SC:(g + 1) * SC],
                            start=(q == 0), stop=(q == DP - 1),
                            perf_mode=DRMODE)
                    nc.scalar.mul(HT[:, dt, g * SC:(g + 1) * SC], ph, KH / KG)

            # ------- A: load fp32, cast fp8, transpose to AT ----
            A_f32 = bigp.tile([P, NTI, D], F32, tag="A_f32")
            A_f8 = bigp.tile([P, NTI, D], F8, tag="A_f8")
            AT = bigp.tile([P, DT, La], F8, tag="AT")
            for g in range(La // SC):
                for k in range(SC // P):
                    t = g * (SC // P) + k
                    nc.sync.dma_start(A_f32[:, t, :], A3[:, t, :])
                    nc.vector.tensor_copy(A_f8[:, t, :], A_f32[:, t, :])
                for dt in range(DT):
                    pst = ps_t.tile([P, SC, 2], F8, tag="ps_t")
                    for k in range(SC // P):
                        it = g * (SC // P) + k
                        nc.tensor.transpose(
                            pst[:, k * P:(k + 1) * P, 0],
                            A_f8[:, it, dt * P:(dt + 1) * P], ident8)
                    nc.scalar.copy(AT[:, dt, g * SC:(g + 1) * SC], pst[:, :, 0])

            # ---- mask row broadcasts + per-partition mask cols ----
            Mbb = constp.tile([P, Lb], BF16, tag="Mbb")
            Mab = constp.tile([P, La], BF16, tag="Mab")
            for mri, L, bc in ((mri_b, Lb, Mbb), (mri_a, La, Mab)):
                mrf = tmp1p.tile([1, LM], BF16, tag="mrow_f")
                nc.vector.tensor_copy(mrf[:, :L], mri[:, :L])
                for c in range(L // SC):
                    pm = ps_t.tile([P, SC], F32, tag="ps_t")
                    nc.tensor.matmul(pm, ones1, mrf[:, c * SC:(c + 1) * SC],
                                     start=True, stop=True)
                    nc.vector.tensor_copy(bc[:, c * SC:(c + 1) * SC], pm)
            mcol_f = constp.tile([P, NTI + NTJ], F32, tag="mcol_f")
            nc.vector.tensor_copy(mcol_f, mcol_i)
            macol = mcol_f[:, :NTI]
            mbcol = mcol_f[:, NTI:]
            # ucol = KS*(1-m)/L  (uniform-softmax weight, KS-prescaled)
            ucol = constp.tile([P, NTI + NTJ], F32, tag="ucol")
            nc.vector.tensor_scalar(
                ucol[:, :NTI], macol, 1.0, -KS / Lb, OP.subtract, OP.mult)
            nc.vector.tensor_scalar(
                ucol[:, NTI:], mbcol, 1.0, -KS / La, OP.subtract, OP.mult)

            # column-replicated KS*(1-m)/L tiles, lhsT for the cA/cB GEMMs
            uac = ucp.tile([P, NTI, P], F8, tag="sc4k8")
            ubc = ucp.tile([P, NTJ, P], F8, tag="sc4k8")
            for t in range(NTI):
                nc.vector.tensor_scalar_mul(uac[:, t, :], onespp, ucol[:, t:t + 1])
            for t in range(NTJ):
                nc.vector.tensor_scalar_mul(
                    ubc[:, t, :], onespp, ucol[:, NTI + t:NTI + t + 1])

            # ------------- cA / cB rank-1 corrections (KS-prescaled) ---------
            cA = constp.tile([P, D], BF16, tag="cA")
            pc = ps_o.tile([P, D], F32, tag="ps_o")
            for pt in range(NPI):
                nc.tensor.matmul(pc, uac[:, 2 * pt:2 * pt + 2, :],
                                 A_f8[:, 2 * pt:2 * pt + 2, :],
                                 start=(pt == 0), stop=(pt == NPI - 1),
                                 perf_mode=DRMODE)
            nc.vector.tensor_copy(cA, pc)
            cB = constp.tile([P, D], BF16, tag="cB")
            pc = ps_o.tile([P, D], F32, tag="ps_o")
            for pt in range(NPJ):
                nc.tensor.matmul(pc, ubc[:, 2 * pt:2 * pt + 2, :],
                                 B_f8[:, 2 * pt:2 * pt + 2, :],
                                 start=(pt == 0), stop=(pt == NPJ - 1),
                                 perf_mode=DRMODE)
            nc.vector.tensor_copy(cB, pc)

            # ======= phase 1: E = mb_j * exp(S)  (i x j), Za row sums =======
            nblk = Lb // SBW
            E = bigp.tile([P, NTI, Lb], F8, tag="E")
            Za = constp.tile([P, NTI], F32, tag="Za")
            for it in range(NTI):
                eraw = erawp.tile([P, Lb], BF16, tag="eraw")
                for blk in range(nblk):
                    ps = ps_s.tile([P, SBW], F32, tag="ps_s")
                    for c in range(SBW // SC):
                        jc = blk * (SBW // SC) + c
                        sl = slice(c * SC, (c + 1) * SC)
                        for q in range(DP):
                            nc.tensor.matmul(
                                ps[:, sl],
                                AT[:, 2 * q:2 * q + 2, it * P:(it + 1) * P],
                                HT[:, 2 * q:2 * q + 2, jc * SC:(jc + 1) * SC],
                                start=(q == 0), stop=(q == DP - 1),
                                perf_mode=DRMODE)
                    nc.scalar.activation(
                        eraw[:, blk * SBW:(blk + 1) * SBW], ps, AF.Exp,
                        scale=1.0 / KH, bias=negc)
                # fused: E = eraw * mb (fp8), Za = rowsum(E) — one DVE pass
                nc.vector.scalar_tensor_tensor(
                    E[:, it, :], eraw, 1.0, Mbb, OP.bypass, OP.mult,
                    accum_out=Za[:, it:it + 1])

            # ---- out_b = E^T @ (A * KS*ma/Za) / KS + cA + B ----
            qa = constp.tile([P, NTI], F32, tag="qa")
            nc.vector.reciprocal(qa, Za)
            nc.vector.scalar_tensor_tensor(qa, qa, KS, macol, OP.mult, OP.mult)
            At_s = bigp.tile([P, NTI, D], F8, tag="ATS")
            for t in range(NTI):
                nc.vector.tensor_scalar_mul(At_s[:, t, :], A_f32[:, t, :],
                                            qa[:, t:t + 1])
            for jt in range(NTJ):
                po = ps_o.tile([P, D], F32, tag="ps_o")
                nc.tensor.matmul(po, ones1, cA[0:1, :], start=True, stop=False)
                for pt in range(NPI):
                    nc.tensor.matmul(po, E[:, 2 * pt:2 * pt + 2,
                                          jt * P:(jt + 1) * P],
                                     At_s[:, 2 * pt:2 * pt + 2, :],
                                     start=False, stop=(pt == NPI - 1),
                                     perf_mode=DRMODE)
                ot = oiop.tile([P, D], F32, tag="io_out")
                nc.vector.scalar_tensor_tensor(
                    ot, po, 1.0 / KS, B_bf[:, jt, :], OP.mult, OP.add)
                nc.sync.dma_start(ob3[:, jt, :], ot)

            # ======= phase 2: E2 = ma_i * exp(S^T)  (j x i), Zb row sums ====
            nblk2 = La // SBW
            E2 = bigp.tile([P, NTJ, La], F8, tag="E")
            Zb = constp.tile([P, NTJ], F32, tag="Zb")
            for jt in range(NTJ):
                eraw = erawp.tile([P, La], BF16, tag="eraw")
                for blk in range(nblk2):
                    ps = ps_s.tile([P, SBW], F32, tag="ps_s")
                    for c in range(SBW // SC):
                        ic = blk * (SBW // SC) + c
                        sl = slice(c * SC, (c + 1) * SC)
                        for q in range(DP):
                            nc.tensor.matmul(
                                ps[:, sl],
                                HT[:, 2 * q:2 * q + 2, jt * P:(jt + 1) * P],
                                AT[:, 2 * q:2 * q + 2, ic * SC:(ic + 1) * SC],
                                start=(q == 0), stop=(q == DP - 1),
                                perf_mode=DRMODE)
                    nc.scalar.activation(
                        eraw[:, blk * SBW:(blk + 1) * SBW], ps, AF.Exp,
                        scale=1.0 / KH, bias=negc)
                nc.vector.scalar_tensor_tensor(
                    E2[:, jt, :], eraw, 1.0, Mab, OP.bypass, OP.mult,
                    accum_out=Zb[:, jt:jt + 1])

            # ---- out_a = E2^T @ (B * KS*mb/Zb) / KS + cB + A ----
            rb = constp.tile([P, NTJ], F32, tag="rb")
            nc.vector.reciprocal(rb, Zb)
            nc.vector.scalar_tensor_tensor(rb, rb, KS, mbcol, OP.mult, OP.mult)
            Bt_s = bigp.tile([P, NTJ, D], F8, tag="ATS")
            for t in range(NTJ):
                nc.vector.tensor_scalar_mul(Bt_s[:, t, :], B_bf[:, t, :],
                                            rb[:, t:t + 1])
            for it in range(NTI):
                po = ps_o.tile([P, D], F32, tag="ps_o")
                nc.tensor.matmul(po, ones1, cB[0:1, :], start=True, stop=False)
                for pt in range(NPJ):
                    nc.tensor.matmul(po, E2[:, 2 * pt:2 * pt + 2,
                                           it * P:(it + 1) * P],
                                     Bt_s[:, 2 * pt:2 * pt + 2, :],
                                     start=False, stop=(pt == NPJ - 1),
                                     perf_mode=DRMODE)
                ot = oiop.tile([P, D], F32, tag="io_out")
                nc.vector.scalar_tensor_tensor(
                    ot, po, 1.0 / KS, A_f32[:, it, :], OP.mult, OP.add)
                nc.sync.dma_start(oa3[:, it, :], ot)

    if split_waits:
        _split_multi_waits(nc)
    return nc


def _split_multi_waits(nc):
    """This toolchain's walrus encodes at most ONE sync wait per engine
    instruction ("Too many sync wait commands"). Hoist all but one wait of
    each offending instruction onto injected same-engine NoOps immediately
    before it: sequential waits on one engine are AND semantics."""
    nop_id = 0
    for bb in nc.main_func.blocks:
        il = bb.instructions
        idx = 0
        while idx < len(il):
            ins = il[idx]
            si = ins.sync_info
            if si is not None and si.on_wait and len(si.on_wait) > 1:
                waits = list(si.on_wait)
                ins.sync_info = mybir.SyncInfo(
                    on_wait=[waits[-1]], on_update=list(si.on_update or []))
                for w in waits[:-1]:
                    nop = mybir.InstNoOp(
                        name=f"I-waitnop-{nop_id}", ins=[], outs=[],
                        engine=ins.engine,
                        sync_info=mybir.SyncInfo(on_wait=[w], on_update=[]))
                    nop_id += 1
                    il.insert(idx, nop)
                    idx += 1
            idx += 1


_NC_CACHE = {}


def _get_nc(La=2048, Lb=2048, D=512):
    key = (La, Lb, D)
    if key not in _NC_CACHE:
        _NC_CACHE[key] = build_nc(La, Lb, D)
    return _NC_CACHE[key]


def _shard(inputs):
    Bn = inputs["input_a"].shape[0]
    names = ("input_a", "input_b", "mask_a", "mask_b")
    in_maps = []
    for b in range(Bn):
        m = {n: np.ascontiguousarray(inputs[n][b]) for n in names}
        m["Wa"] = np.ascontiguousarray(inputs["Wa"])
        m["Wb"] = np.ascontiguousarray(inputs["Wb"])
        in_maps.append(m)
    return in_maps


def kernel(**inputs):
    from concourse.bass_utils import run_bass_kernel_spmd

    inputs = {k: np.asarray(v) for k, v in inputs.items()}
    # the kernel folds the (identically-zero) biases away
    assert not inputs["ba"].any() and not inputs["bb"].any()
    Bn, La, D = inputs["input_a"].shape
    Lb = inputs["input_b"].shape[1]
    nc = _get_nc(La, Lb, D)
    in_maps = _shard(inputs)
    res = run_bass_kernel_spmd(nc, in_maps, core_ids=list(range(Bn))).results
    out_a = np.stack([res[b]["out_a"] for b in range(Bn)])
    out_b = np.stack([res[b]["out_b"] for b in range(Bn)])
    return out_a, out_b
